# revision 7
# baseline (speedup 1.0000x reference)
"""ComplexPolarAttention Trainium2 kernel.

score_ij = sum_d mag_i,d mag_j,d cos(phase_i,d - phase_j,d)
         = a_i . a_j + b_i . b_j          with a = mag*cos(phase), b = mag*sin(phase)
out_mag   = softmax(score, axis=1) @ mag
out_phase = softmax(score, axis=1) @ phase

Strategy (8 NeuronCores, SPMD, no collectives):
  - Rows (queries) sharded; keys replicated. Per-core inputs are ROTATED
    along the key axis so that core c's queries are always columns 0..q of
    its own key panel (softmax over keys is permutation invariant), so the
    query operand is just a slice of the key panel.
  - The packed ab^T = [a|b]^T [128=2D, N] panel fuses the two score GEMMs
    into ONE K=128 fp32r matmul per key block of 128; scores are computed
    transposed, S^T[k_blk=128, q] in PSUM, then exp'd in one wide
    [128, 1024] ACTIVATE per key block (scores < 88 so exp can't overflow).
  - Value matmuls use ONE packed stationary [mag|phase] (128 cols) per key
    block, so es streams through the PE once per block (2x512 cols), not
    twice: numerators for new_mag AND new_phase come out of the same pass
    as psum rows [0:64]=mag^T, [64:128]=phase^T, accumulated over all 64
    key blocks into two [128, 512] PSUM banks (one per q half).
  - The softmax denominator comes from es_sum = sum_kb es (running DVE
    tensor_add, the vector engine is otherwise idle), reduced over keys at
    the very end by a single ones^T [128,1] matmul per q half.
  - The final divide happens on host during the gather.
  - All DRAM inputs are chunk-major so every dma_start reads one fully
    contiguous block; the ab^T chunks ride the sync HWDGE queue, the value
    matrix the gpsimd SWDGE queue, so the k-loop's critical first chunk
    lands as early as possible and later chunks stream in under compute.
"""

import numpy as np
from contextlib import ExitStack

import concourse.bass as bass
import concourse.tile as tile
from concourse import bacc, mybir
from concourse.bass_utils import run_bass_kernel_spmd

F32 = mybir.dt.float32
F32R = mybir.dt.float32r


def abt_chunk_widths(n):
    widths, rem = [], n
    for w in (512, 512):
        if rem >= w:
            widths.append(w)
            rem -= w
    while rem:
        w = min(1024, rem)
        widths.append(w)
        rem -= w
    return widths


def build_program(n=8192, d=64, n_cores=8, enable_asserts=False):
    """Build the SPMD Bass program. Every core runs identical IR; per-core
    behavior comes only from per-core (rotated) input data."""
    assert d == 64
    q = n // n_cores            # queries per core
    kblocks = n // 128          # key blocks of 128
    qblk = q // 2               # half processed per matmul (fp32 moving max 512)
    assert qblk <= 512 and n % 128 == 0

    nc = bacc.Bacc(
        "TRN2",
        target_bir_lowering=False,
        debug=False,
        enable_asserts=enable_asserts,
        num_devices=n_cores,
    )

    # ---- DRAM I/O (all per-core arrays rotated so queries = keys[0:q]) ----
    chunks = abt_chunk_widths(n)
    vchunk = max(1, kblocks // 16)
    nvch = kblocks // vchunk
    # packed [a|b]^T panel, one DRAM tensor per chunk; the first chunks are
    # small so the score stream can start sooner
    abt_in = [nc.dram_tensor(f"abt{i}", [128, w], F32R,
                             kind="ExternalInput").ap()
              for i, w in enumerate(chunks)]
    # packed [mag | phase] value matrix, chunk-major [nvch, 128, vchunk*128]
    vt = nc.dram_tensor("vt", [nvch, 128, vchunk * 128], F32R,
                        kind="ExternalInput").ap()
    ones_in = nc.dram_tensor("onesv", [128, 1], F32R,
                             kind="ExternalInput").ap()

    onum = nc.dram_tensor("onum", [128, q], F32, kind="ExternalOutput").ap()
    oden = nc.dram_tensor("oden", [1, q], F32, kind="ExternalOutput").ap()

    with tile.TileContext(nc) as tc, ExitStack() as ctx:
        persist = ctx.enter_context(tc.tile_pool(name="persist", bufs=1))
        epool = ctx.enter_context(tc.tile_pool(name="exps", bufs=5))
        opool = ctx.enter_context(tc.tile_pool(name="outs", bufs=4))
        spool = ctx.enter_context(tc.tile_pool(name="scores", bufs=3, space="PSUM"))
        apool = ctx.enter_context(tc.tile_pool(name="accum", bufs=1, space="PSUM"))

        abt = persist.tile([128, n], F32R)       # [a|b]^T for all keys
        vt_t = persist.tile([128, kblocks, 128], F32R)
        ones = persist.tile([128, 1], F32R)
        es_sum = persist.tile([128, q], F32R)

        # ab^T chunks on the sync queue -- chunk 0 gates the first matmul
        off = 0
        for i, w in enumerate(chunks):
            nc.sync.dma_start(out=abt[:, off:off + w], in_=abt_in[i])
            off += w
        abq = abt[:, 0:q]          # queries are the first q key columns

        # value matrix on the gpsimd queue, fine-grained and interleaved
        # so the first key blocks' stationaries land just after exp0
        for vi in range(nvch):
            b0 = vi * vchunk
            b1 = b0 + vchunk
            nc.gpsimd.dma_start(out=vt_t[:, b0:b1, :], in_=vt[vi, :, :])

        nc.sync.dma_start(out=ones[:, :], in_=ones_in)

        # ---- PSUM budget: scores [128,1024]x3bufs = 6 banks, accA/accB = 2.
        accA = apool.tile([128, qblk], F32, name="accA", tag="accA")
        accB = apool.tile([128, qblk], F32, name="accB", tag="accB")

        def value_mms(es, kb):
            first, last = (kb == 0), (kb == kblocks - 1)
            nc.tensor.matmul(out=accA[:, :], lhsT=vt_t[:, kb, :],
                             rhs=es[:, 0:qblk], start=first, stop=last)
            nc.tensor.matmul(out=accB[:, :], lhsT=vt_t[:, kb, :],
                             rhs=es[:, qblk:q], start=first, stop=last)
            if first:
                nc.vector.tensor_copy(es_sum[:, :], es[:, :])
            else:
                nc.vector.tensor_add(es_sum[:, :], es_sum[:, :], es[:, :])

        # warm the PE clock (HAM) during the head DMA window: junk fp32
        # matmuls on zeros (4 cyc/col) so the real stream starts at 2.4 GHz
        wsrc = persist.tile([128, 512], F32)
        nc.vector.memset(wsrc[:, :], 0.0)
        warm = spool.tile([128, q], F32, name="warm", tag="ss")
        for _ in range(2):      # ~1.7us each cold
            nc.tensor.matmul(out=warm[0:16, 0:512], lhsT=wsrc[:, 0:16],
                             rhs=wsrc[:, 0:512], start=True, stop=True)

        es_hist = []
        for kb in range(kblocks):
            if len(es_hist) >= 3:
                value_mms(es_hist[-3], kb - 3)
            ss = spool.tile([128, q], F32)
            for j in range(2):
                nc.tensor.matmul(
                    out=ss[:, j * qblk:(j + 1) * qblk],
                    lhsT=abt[:, kb * 128:(kb + 1) * 128],
                    rhs=abq[:, j * qblk:(j + 1) * qblk],
                    start=True, stop=True,
                )
            es = epool.tile([128, q], F32R)
            nc.scalar.activation(
                es[:, :], ss[:, :], mybir.ActivationFunctionType.Exp,
            )
            es_hist.append(es)
        value_mms(es_hist[-3], kblocks - 3)
        value_mms(es_hist[-2], kblocks - 2)
        value_mms(es_hist[-1], kblocks - 1)

        # denominator: ones^T [128,1] @ es_sum -> [1, qblk] per half
        psD = []
        for j in range(2):
            pd = spool.tile([1, qblk], F32, name=f"psD{j}", tag="ss")
            nc.tensor.matmul(out=pd[:, :], lhsT=ones[:, :],
                             rhs=es_sum[:, j * qblk:(j + 1) * qblk],
                             start=True, stop=True)
            psD.append(pd)

        # outputs: PSUM -> SBUF (DVE for half 0, ACT for half 1) -> DRAM
        oA = opool.tile([128, qblk], F32, tag="oA")
        nc.vector.tensor_copy(oA[:, :], accA[:, :])
        nc.sync.dma_start(out=onum[:, 0:qblk], in_=oA[:, :])
        oB = opool.tile([128, qblk], F32, tag="oB")
        nc.scalar.activation(oB[:, :], accB[:, :],
                             mybir.ActivationFunctionType.Copy)
        nc.sync.dma_start(out=onum[:, qblk:q], in_=oB[:, :])
        for j in range(2):
            od = opool.tile([1, qblk], F32, tag=f"oD{j}")
            eng = nc.vector.tensor_copy if j == 0 else None
            if eng is not None:
                eng(od[:, :], psD[j][:, :])
            else:
                nc.scalar.activation(od[:, :], psD[j][:, :],
                                     mybir.ActivationFunctionType.Copy)
            nc.sync.dma_start(out=oden[:, j * qblk:(j + 1) * qblk],
                              in_=od[:, :])

    nc.compile()
    return nc


def make_inputs(mag, phase, n_cores=8):
    """Host-side sharding/layout prep -> per-core (key-rotated) input maps."""
    n, d = mag.shape
    q = n // n_cores
    kblocks = n // 128
    mag = np.ascontiguousarray(mag, dtype=np.float32)
    phase = np.ascontiguousarray(phase, dtype=np.float32)

    a = mag * np.cos(phase)
    b = mag * np.sin(phase)
    abt_g = np.concatenate([a.T, b.T], axis=0).astype(np.float32)  # [128, n]
    v_nat = np.concatenate([mag, phase], axis=1)                   # [n, 128]

    chunks = abt_chunk_widths(n)
    vchunk = max(1, kblocks // 16)
    nvch = kblocks // vchunk

    def tile_nat(x):  # [n, m] -> [nvch, 128, vchunk*m] chunk-major
        m = x.shape[1]
        y = x.reshape(nvch, vchunk, 128, m).transpose(0, 2, 1, 3)
        return np.ascontiguousarray(y.reshape(nvch, 128, vchunk * m))

    in_maps = []
    for c in range(n_cores):
        r = c * q
        abt_c = np.roll(abt_g, -r, axis=1)
        m = {"vt": tile_nat(np.roll(v_nat, -r, axis=0)),
             "onesv": np.ones((128, 1), np.float32)}
        off = 0
        for i, w in enumerate(chunks):
            m[f"abt{i}"] = np.ascontiguousarray(abt_c[:, off:off + w])
            off += w
        in_maps.append(m)
    return in_maps


def gather_outputs(results, n, d, n_cores=8):
    """Per-core [128,q] transposed unnormalized sums + [1,q] denominators
    -> full outputs."""
    new_mag = np.empty((n, d), np.float32)
    new_phase = np.empty((n, d), np.float32)
    q = n // n_cores
    for c in range(n_cores):
        onum = results[c]["onum"]      # [128, q]
        den = results[c]["oden"]       # [1, q]
        qsl = slice(c * q, (c + 1) * q)
        new_mag[qsl] = (onum[:64, :] / den).T
        new_phase[qsl] = (onum[64:128, :] / den).T
    return new_mag, new_phase


_PROGRAM_CACHE = {}


def _get_program(n, d, n_cores):
    key = (n, d, n_cores)
    if key not in _PROGRAM_CACHE:
        _PROGRAM_CACHE[key] = build_program(n=n, d=d, n_cores=n_cores)
    return _PROGRAM_CACHE[key]


def kernel(mag, phase):
    mag = np.asarray(mag, dtype=np.float32)
    phase = np.asarray(phase, dtype=np.float32)
    n, d = mag.shape
    n_cores = 8
    nc = _get_program(n, d, n_cores)
    in_maps = make_inputs(mag, phase, n_cores=n_cores)
    res = run_bass_kernel_spmd(nc, in_maps, list(range(n_cores)))
    return gather_outputs(res.results, n, d, n_cores=n_cores)
